# revision 36
# baseline (speedup 1.0000x reference)
"""Window-routed sparse attention on 8 TRN2 NeuronCores.

Sharding: 64 windows x 8 cores = 8 windows/core, processed as 4 pairs of 2
windows packed into the 128 partitions. Host precomputes the tiny routing
path (region means, a_r [64,64]) and the window-mixed q_m/k_m in fp32; each
core runs the heavy windowed attention relu(q_m k_m^T) v on the Tensor
engine in bf16:

- QK matmuls: contract dim c=64, so two windows run concurrently as row
  tiles of the PE array (window A in array rows 0-63, window B in 64-127).
  QK output is written to PSUM as bf16 (no accumulation needed), which
  halves PSUM footprint and doubles VectorE evacuation throughput.
- relu evacuation PSUM->SBUF is the second roofline wall (only ScalarE and
  VectorE can read PSUM, ~1 elem/cycle/lane fp32, 2/cycle bf16 on DVE);
  tiles are split between both engines.
- AV matmuls: out dim c=64, so two windows run concurrently as col tiles
  (window A -> out partitions 0-63, B -> 64-127), accumulating over the 8
  s-chunks in fp32 PSUM. Separate banks per accumulation group.
"""

import sys

sys.path.insert(0, "/opt/trn_rl_repo")

import numpy as np
import ml_dtypes

C = 64          # channels
NW = 64         # windows (8x8 grid of 32x32 patches on 256x256)
T = 1024        # tokens per window (32*32)
NCORES = 8
WPC = NW // NCORES   # windows per core
NPAIR = WPC // 2     # window pairs per core
NCH = 8              # s-chunks of 128 tokens

_CACHE = {}

# Debug knobs (harness just calls kernel(); test.py flips TRACE for profiling)
TRACE = False
TRACE_DIR = None
LAST = None


def _build_program():
    import concourse.mybir as mybir
    from concourse import bacc
    from concourse.tile import TileContext

    bf16 = mybir.dt.bfloat16
    f32 = mybir.dt.float32
    Relu = mybir.ActivationFunctionType.Relu

    nc = bacc.Bacc(None, target_bir_lowering=False)
    # [pair, 2c, t] channel-major; window A = partitions 0:64, B = 64:128
    qm_d = nc.declare_dram_parameter("qm", [NPAIR, 128, T], bf16, isOutput=False)
    km_d = nc.declare_dram_parameter("km", [NPAIR, 128, T], bf16, isOutput=False)
    # [pair, s_in_chunk, window, chunk, c]
    v_d = nc.declare_dram_parameter("v", [NPAIR, 128, 2, NCH, C], bf16, isOutput=False)
    o_d = nc.declare_dram_parameter("o", [NPAIR, 128, T], f32, isOutput=True)

    with TileContext(nc) as tc:
        with (
            tc.tile_pool(name="io", bufs=2) as io_pool,
            tc.tile_pool(name="at", bufs=4) as attn_pool,
            tc.tile_pool(name="ob", bufs=2) as o_pool,
            tc.tile_pool(name="pq", bufs=3, space="PSUM") as pq_pool,
            tc.tile_pool(name="po", bufs=1, space="PSUM") as po_pool,
        ):
            # PE warmup burst: ~12 dense matmuls on scratch data while the
            # first input DMAs are in flight, so the HAM un-throttles the PE
            # clock (4/8 -> 8/8) before the real stream starts.
            wsrc = io_pool.tile([128, 512], bf16, tag="wsrc", bufs=1)
            wps = pq_pool.tile([128, 512], f32, tag="qk", name="wps")
            nc.vector.memset(wsrc, 0.0)
            for _ in range(6):
                nc.tensor.matmul(
                    out=wps, lhsT=wsrc[:, 0:128], rhs=wsrc,
                    start=True, stop=True,
                )

            # Single flat chunk stream across all pairs: the AV lag queue
            # carries over pair boundaries, so the PE never drains while a
            # pair's tail evacuations finish.
            pends = []  # (attn_a, attn_b, k, v_t, ps_o, p) awaiting AV

            def pop_pend():
                attn_a, attn_b, k, v_tp, ps_op, pp = pends.pop(0)
                _issue_av(nc, v_tp, ps_op, attn_a, attn_b, k)
                if k == NCH - 1:
                    # pair pp's accumulation is complete: evacuate + store
                    # (single FD=1024 copy, alternating engines per pair;
                    # the last pair splits across both engines + two DMAs
                    # to shorten the kernel tail)
                    o_t = o_pool.tile([128, T], f32, tag="o")
                    if pp == NPAIR - 1:
                        nc.scalar.copy(out=o_t[:, 0:512], in_=ps_op[:, 0:512])
                        nc.sync.dma_start(out=o_d[pp, :, 0:512], in_=o_t[:, 0:512])
                        nc.vector.tensor_copy(out=o_t[:, 512:1024], in_=ps_op[:, 512:1024])
                        nc.gpsimd.dma_start(out=o_d[pp, :, 512:1024], in_=o_t[:, 512:1024])
                    elif pp % 2 == 0:
                        nc.scalar.copy(out=o_t, in_=ps_op)
                        nc.sync.dma_start(out=o_d[pp], in_=o_t)
                    else:
                        nc.vector.tensor_copy(out=o_t, in_=ps_op)
                        nc.sync.dma_start(out=o_d[pp], in_=o_t)

            for p in range(NPAIR):
                qm_t = io_pool.tile([128, T], bf16, tag="qm")
                km_t = io_pool.tile([128, T], bf16, tag="km")
                v_t = io_pool.tile([128, 2, NCH, C], bf16, tag="v")
                # Split input DMAs so the first QK only waits on the first
                # chunks of km and the full qm; alternate sync/gpsimd DMA
                # queues so descriptor generation runs in parallel.
                nc.sync.dma_start(out=km_t[:, 0:512], in_=km_d[p, :, 0:512])
                nc.gpsimd.dma_start(out=qm_t[:, 0:512], in_=qm_d[p, :, 0:512])
                nc.gpsimd.dma_start(out=qm_t[:, 512:1024], in_=qm_d[p, :, 512:1024])
                nc.sync.dma_start(out=km_t[:, 512:1024], in_=km_d[p, :, 512:1024])
                nc.gpsimd.dma_start(out=v_t, in_=v_d[p])

                # o accumulator: [128,1024] = 2 banks; each bank holds both
                # windows (A partitions 0-63, B 64-127). has_written is per
                # element, so each window opens its own group at k=0.
                ps_o = po_pool.tile([128, T], f32, tag="o", name="ps_o")

                for k in range(NCH):
                    # Issue order inside a chunk: QK_A, AV(k-2), QK_B. The
                    # psum ring (3 tiles over 2 allocs/chunk) makes QK_B(k+1)
                    # wait on relu_A(k); issuing B last gives that edge two
                    # pair-issues of PE work to resolve behind, instead of
                    # stalling the whole QK pair. B(k) then pairs with
                    # A(k+1) in the array (disjoint row groups).
                    ps_a = pq_pool.tile([128, T], f32, tag="qk", name="ps_a")
                    for h in range(2):
                        ts = slice(h * 512, (h + 1) * 512)
                        nc.tensor.matmul(
                            out=ps_a[:, ts],
                            lhsT=km_t[0:64, k * 128:(k + 1) * 128],
                            rhs=qm_t[0:64, ts],
                            start=True, stop=True,
                        )
                    attn_a = attn_pool.tile([128, T], bf16, tag="aA", name="attn_a")
                    nc.scalar.activation(out=attn_a, in_=ps_a, func=Relu)

                    if len(pends) >= 2:
                        pop_pend()

                    ps_b = pq_pool.tile([128, T], f32, tag="qk", name="ps_b")
                    for h in range(2):
                        ts = slice(h * 512, (h + 1) * 512)
                        nc.tensor.matmul(
                            out=ps_b[:, ts],
                            lhsT=km_t[64:128, k * 128:(k + 1) * 128],
                            rhs=qm_t[64:128, ts],
                            start=True, stop=True,
                        )
                    attn_b = attn_pool.tile([128, T], bf16, tag="aB", name="attn_b")
                    nc.vector.tensor_scalar_max(attn_b, ps_b, 0.0)

                    pends.append((attn_a, attn_b, k, v_t, ps_o, p))

            while pends:
                pop_pend()

    nc.finalize()
    return nc


def _issue_av(nc, v_t, ps_o, attn_a, attn_b, k):
    """AV matmuls for chunk k: two windows concurrently as PE col tiles."""
    last = k == NCH - 1
    for h in range(2):
        ts = slice(h * 512, (h + 1) * 512)
        nc.tensor.matmul(
            out=ps_o[0:64, ts],
            lhsT=v_t[:, 0, k, :],
            rhs=attn_a[:, ts],
            start=(k == 0), stop=last,
        )
        nc.tensor.matmul(
            out=ps_o[64:128, ts],
            lhsT=v_t[:, 1, k, :],
            rhs=attn_b[:, ts],
            start=(k == 0), stop=last,
            skip_group_check=True,
        )


def kernel(x, W, bias):
    from concourse.bass_utils import run_bass_kernel_spmd

    x = np.asarray(x, dtype=np.float32)
    W = np.asarray(W, dtype=np.float32)
    bias = np.asarray(bias, dtype=np.float32)

    # ---- host prep: windows, qkv, routing, mixing (tiny vs attention) ----
    # xw: [nw, T, c]
    xw = (
        x.reshape(C, 8, 32, 8, 32)
        .transpose(1, 3, 2, 4, 0)
        .reshape(NW, T, C)
    )
    qkv = xw @ W.T + bias  # [nw, T, 3c]
    q, k, v = qkv[..., :C], qkv[..., C:2 * C], qkv[..., 2 * C:]
    q_r = q.mean(axis=1)  # [nw, c]
    k_r = k.mean(axis=1)
    a_r = np.maximum(q_r @ k_r.T, 0.0)  # [nw, nw]
    k_m = np.tensordot(a_r, k, axes=(1, 0))  # [nw, T, c]
    q_m = np.tensordot(a_r, q, axes=(1, 0))

    if "nc" not in _CACHE:
        _CACHE["nc"] = _build_program()
    nc = _CACHE["nc"]

    bf16 = ml_dtypes.bfloat16
    in_maps = []
    for m in range(NCORES):
        s = slice(m * WPC, (m + 1) * WPC)
        # [wpc, c, T] -> [npair, 128, T]
        qm_c = np.ascontiguousarray(
            q_m[s].transpose(0, 2, 1).reshape(NPAIR, 128, T).astype(bf16)
        )
        km_c = np.ascontiguousarray(
            k_m[s].transpose(0, 2, 1).reshape(NPAIR, 128, T).astype(bf16)
        )
        # [wpc, T, c] -> [npair, 2, chunk, 128, c] -> [npair, 128, 2, chunk, c]
        v_c = np.ascontiguousarray(
            v[s].reshape(NPAIR, 2, NCH, 128, C).transpose(0, 3, 1, 2, 4).astype(bf16)
        )
        in_maps.append({"qm": qm_c, "km": km_c, "v": v_c})

    global LAST
    kw = {}
    if TRACE:
        kw = dict(trace=True, tmpdir=TRACE_DIR)
    res = run_bass_kernel_spmd(nc, in_maps, list(range(NCORES)), **kw)
    LAST = res

    # [npair, 128, T] -> [wpc, c, T] per core; concat to [c, nw, T]
    outs = [
        res.results[m]["o"].reshape(WPC, C, T).transpose(1, 0, 2)
        for m in range(NCORES)
    ]
    o_cm = np.concatenate(outs, axis=1)  # [c, nw, T]

    # fold back: [c, jh, jw, th, tw] -> [1, c, 256, 256]
    o_img = (
        o_cm.reshape(C, 8, 8, 32, 32)
        .transpose(0, 1, 3, 2, 4)
        .reshape(1, C, 256, 256)
    )
    return o_img.astype(np.float32)


# revision 37
# speedup vs baseline: 1.0387x; 1.0387x over previous
"""Window-routed sparse attention on 8 TRN2 NeuronCores.

Sharding: 64 windows x 8 cores = 8 windows/core, processed as 4 pairs of 2
windows packed into the 128 partitions. Host precomputes the tiny routing
path (region means, a_r [64,64]) and the window-mixed q_m/k_m in fp32; each
core runs the heavy windowed attention relu(q_m k_m^T) v on the Tensor
engine in bf16:

- QK matmuls: contract dim c=64, so two windows run concurrently as row
  tiles of the PE array (window A in array rows 0-63, window B in 64-127).
  QK output is written to PSUM as bf16 (no accumulation needed), which
  halves PSUM footprint and doubles VectorE evacuation throughput.
- relu evacuation PSUM->SBUF is the second roofline wall (only ScalarE and
  VectorE can read PSUM, ~1 elem/cycle/lane fp32, 2/cycle bf16 on DVE);
  tiles are split between both engines.
- AV matmuls: out dim c=64, so two windows run concurrently as col tiles
  (window A -> out partitions 0-63, B -> 64-127), accumulating over the 8
  s-chunks in fp32 PSUM. Separate banks per accumulation group.
"""

import sys

sys.path.insert(0, "/opt/trn_rl_repo")

import numpy as np
import ml_dtypes

C = 64          # channels
NW = 64         # windows (8x8 grid of 32x32 patches on 256x256)
T = 1024        # tokens per window (32*32)
NCORES = 8
WPC = NW // NCORES   # windows per core
NPAIR = WPC // 2     # window pairs per core
NCH = 8              # s-chunks of 128 tokens

_CACHE = {}

# Debug knobs (harness just calls kernel(); test.py flips TRACE for profiling)
TRACE = False
TRACE_DIR = None
LAST = None


def _build_program():
    import concourse.mybir as mybir
    from concourse import bacc
    from concourse.tile import TileContext

    bf16 = mybir.dt.bfloat16
    f32 = mybir.dt.float32
    Relu = mybir.ActivationFunctionType.Relu

    nc = bacc.Bacc(None, target_bir_lowering=False)
    # [pair, 2c, t] channel-major; window A = partitions 0:64, B = 64:128
    qm_d = nc.declare_dram_parameter("qm", [NPAIR, 128, T], bf16, isOutput=False)
    km_d = nc.declare_dram_parameter("km", [NPAIR, 128, T], bf16, isOutput=False)
    # [pair, s_in_chunk, window, chunk, c]
    v_d = nc.declare_dram_parameter("v", [NPAIR, 128, 2, NCH, C], bf16, isOutput=False)
    o_d = nc.declare_dram_parameter("o", [NPAIR, 128, T], f32, isOutput=True)

    with TileContext(nc) as tc:
        with (
            tc.tile_pool(name="io", bufs=2) as io_pool,
            tc.tile_pool(name="at", bufs=3) as attn_pool,
            tc.tile_pool(name="ob", bufs=2) as o_pool,
            tc.tile_pool(name="pq", bufs=3, space="PSUM") as pq_pool,
            tc.tile_pool(name="po", bufs=1, space="PSUM") as po_pool,
        ):
            # PE warmup burst: ~12 dense matmuls on scratch data while the
            # first input DMAs are in flight, so the HAM un-throttles the PE
            # clock (4/8 -> 8/8) before the real stream starts.
            wsrc = io_pool.tile([128, 512], bf16, tag="wsrc", bufs=1)
            wps = pq_pool.tile([128, 512], f32, tag="qk", name="wps")
            nc.vector.memset(wsrc, 0.0)
            for _ in range(6):
                nc.tensor.matmul(
                    out=wps, lhsT=wsrc[:, 0:128], rhs=wsrc,
                    start=True, stop=True,
                )

            # Single flat chunk stream across all pairs: the AV lag queue
            # carries over pair boundaries, so the PE never drains while a
            # pair's tail evacuations finish.
            pends = []  # (attn_a, attn_b, k, v_t, ps_o, p) awaiting AV

            def pop_pend():
                attn_a, attn_b, k, v_tp, ps_op, pp = pends.pop(0)
                _issue_av(nc, v_tp, ps_op, attn_a, attn_b, k)
                if k == NCH - 1:
                    # pair pp's accumulation is complete: evacuate + store
                    # (single FD=1024 copy, alternating engines per pair;
                    # the last pair splits across both engines + two DMAs
                    # to shorten the kernel tail)
                    o_t = o_pool.tile([128, T], f32, tag="o")
                    if pp == NPAIR - 1:
                        nc.scalar.copy(out=o_t[:, 0:512], in_=ps_op[:, 0:512])
                        nc.sync.dma_start(out=o_d[pp, :, 0:512], in_=o_t[:, 0:512])
                        nc.vector.tensor_copy(out=o_t[:, 512:1024], in_=ps_op[:, 512:1024])
                        nc.gpsimd.dma_start(out=o_d[pp, :, 512:1024], in_=o_t[:, 512:1024])
                    elif pp % 2 == 0:
                        nc.scalar.copy(out=o_t, in_=ps_op)
                        nc.sync.dma_start(out=o_d[pp], in_=o_t)
                    else:
                        nc.vector.tensor_copy(out=o_t, in_=ps_op)
                        nc.sync.dma_start(out=o_d[pp], in_=o_t)

            for p in range(NPAIR):
                qm_t = io_pool.tile([128, T], bf16, tag="qm")
                km_t = io_pool.tile([128, T], bf16, tag="km")
                v_t = io_pool.tile([128, 2, NCH, C], bf16, tag="v")
                # Split input DMAs so the first QK only waits on the first
                # chunks of km and the full qm; alternate sync/gpsimd DMA
                # queues so descriptor generation runs in parallel.
                nc.sync.dma_start(out=km_t[:, 0:512], in_=km_d[p, :, 0:512])
                nc.gpsimd.dma_start(out=qm_t[:, 0:512], in_=qm_d[p, :, 0:512])
                nc.gpsimd.dma_start(out=qm_t[:, 512:1024], in_=qm_d[p, :, 512:1024])
                nc.sync.dma_start(out=km_t[:, 512:1024], in_=km_d[p, :, 512:1024])
                nc.gpsimd.dma_start(out=v_t, in_=v_d[p])

                # o accumulator: [128,1024] = 2 banks; each bank holds both
                # windows (A partitions 0-63, B 64-127). has_written is per
                # element, so each window opens its own group at k=0.
                ps_o = po_pool.tile([128, T], f32, tag="o", name="ps_o")

                for k in range(NCH):
                    # Issue order inside a chunk: QK_A, AV(k-2), QK_B. The
                    # psum ring (3 tiles over 2 allocs/chunk) makes QK_B(k+1)
                    # wait on relu_A(k); issuing B last gives that edge two
                    # pair-issues of PE work to resolve behind, instead of
                    # stalling the whole QK pair. B(k) then pairs with
                    # A(k+1) in the array (disjoint row groups).
                    ps_a = pq_pool.tile([128, T], f32, tag="qk", name="ps_a")
                    for h in range(2):
                        ts = slice(h * 512, (h + 1) * 512)
                        nc.tensor.matmul(
                            out=ps_a[:, ts],
                            lhsT=km_t[0:64, k * 128:(k + 1) * 128],
                            rhs=qm_t[0:64, ts],
                            start=True, stop=True,
                        )
                    attn_a = attn_pool.tile([128, T], bf16, tag="aA", name="attn_a")
                    nc.scalar.activation(out=attn_a, in_=ps_a, func=Relu)

                    if len(pends) >= 2:
                        pop_pend()

                    ps_b = pq_pool.tile([128, T], f32, tag="qk", name="ps_b")
                    for h in range(2):
                        ts = slice(h * 512, (h + 1) * 512)
                        nc.tensor.matmul(
                            out=ps_b[:, ts],
                            lhsT=km_t[64:128, k * 128:(k + 1) * 128],
                            rhs=qm_t[64:128, ts],
                            start=True, stop=True,
                        )
                    attn_b = attn_pool.tile([128, T], bf16, tag="aB", name="attn_b")
                    nc.vector.tensor_scalar_max(attn_b, ps_b, 0.0)

                    pends.append((attn_a, attn_b, k, v_t, ps_o, p))

            while pends:
                pop_pend()

    nc.finalize()
    return nc


def _issue_av(nc, v_t, ps_o, attn_a, attn_b, k):
    """AV matmuls for chunk k: two windows concurrently as PE col tiles."""
    last = k == NCH - 1
    for h in range(2):
        ts = slice(h * 512, (h + 1) * 512)
        nc.tensor.matmul(
            out=ps_o[0:64, ts],
            lhsT=v_t[:, 0, k, :],
            rhs=attn_a[:, ts],
            start=(k == 0), stop=last,
        )
        nc.tensor.matmul(
            out=ps_o[64:128, ts],
            lhsT=v_t[:, 1, k, :],
            rhs=attn_b[:, ts],
            start=(k == 0), stop=last,
            skip_group_check=True,
        )


def kernel(x, W, bias):
    from concourse.bass_utils import run_bass_kernel_spmd

    x = np.asarray(x, dtype=np.float32)
    W = np.asarray(W, dtype=np.float32)
    bias = np.asarray(bias, dtype=np.float32)

    # ---- host prep: windows, qkv, routing, mixing (tiny vs attention) ----
    # xw: [nw, T, c]
    xw = (
        x.reshape(C, 8, 32, 8, 32)
        .transpose(1, 3, 2, 4, 0)
        .reshape(NW, T, C)
    )
    qkv = xw @ W.T + bias  # [nw, T, 3c]
    q, k, v = qkv[..., :C], qkv[..., C:2 * C], qkv[..., 2 * C:]
    q_r = q.mean(axis=1)  # [nw, c]
    k_r = k.mean(axis=1)
    a_r = np.maximum(q_r @ k_r.T, 0.0)  # [nw, nw]
    k_m = np.tensordot(a_r, k, axes=(1, 0))  # [nw, T, c]
    q_m = np.tensordot(a_r, q, axes=(1, 0))

    if "nc" not in _CACHE:
        _CACHE["nc"] = _build_program()
    nc = _CACHE["nc"]

    bf16 = ml_dtypes.bfloat16
    in_maps = []
    for m in range(NCORES):
        s = slice(m * WPC, (m + 1) * WPC)
        # [wpc, c, T] -> [npair, 128, T]
        qm_c = np.ascontiguousarray(
            q_m[s].transpose(0, 2, 1).reshape(NPAIR, 128, T).astype(bf16)
        )
        km_c = np.ascontiguousarray(
            k_m[s].transpose(0, 2, 1).reshape(NPAIR, 128, T).astype(bf16)
        )
        # [wpc, T, c] -> [npair, 2, chunk, 128, c] -> [npair, 128, 2, chunk, c]
        v_c = np.ascontiguousarray(
            v[s].reshape(NPAIR, 2, NCH, 128, C).transpose(0, 3, 1, 2, 4).astype(bf16)
        )
        in_maps.append({"qm": qm_c, "km": km_c, "v": v_c})

    global LAST
    kw = {}
    if TRACE:
        kw = dict(trace=True, tmpdir=TRACE_DIR)
    res = run_bass_kernel_spmd(nc, in_maps, list(range(NCORES)), **kw)
    LAST = res

    # [npair, 128, T] -> [wpc, c, T] per core; concat to [c, nw, T]
    outs = [
        res.results[m]["o"].reshape(WPC, C, T).transpose(1, 0, 2)
        for m in range(NCORES)
    ]
    o_cm = np.concatenate(outs, axis=1)  # [c, nw, T]

    # fold back: [c, jh, jw, th, tw] -> [1, c, 256, 256]
    o_img = (
        o_cm.reshape(C, 8, 8, 32, 32)
        .transpose(0, 1, 3, 2, 4)
        .reshape(1, C, 256, 256)
    )
    return o_img.astype(np.float32)
